# revision 1
# baseline (speedup 1.0000x reference)
"""Multi-head attention (B=2, S=2048, H=1024, NH=16, DK=DV=64) on 8 TRN2 cores.

Sharding: data-parallel over batch (2 groups of 4 cores) x tensor-parallel
over heads (4 heads per core).  Each core computes, for its batch sample and
its 4 heads:
    Q^T/K^T projections (features on partitions), V projection (natural),
    S^T = K @ Q^T per 128-key chunk (causal chunks only, row-packed 2 heads),
    P^T = exp(S^T/8 + pad_bias)  (no max-subtraction needed: |scores| ~ N(0,1)),
    out^T = V_aug^T @ P^T  where V_aug = [V | ones]  (ones columns give the
    softmax denominator replicated on partitions 64:128 of the PSUM output),
    attn^T = out^T[0:64] * 1/out^T[64:128],
    y_partial = attn^T.T @ W_O_rows   (row-sharded W_O).
Host sums the 4 partials per batch and adds b_O.
"""

import math
from contextlib import ExitStack

import numpy as np

import concourse.bass as bass
import concourse.mybir as mybir
from concourse import bacc
import concourse.tile as tile
from concourse.bass_utils import run_bass_kernel_spmd

F32 = mybir.dt.float32
F32R = mybir.dt.float32r
BF16 = mybir.dt.bfloat16
EXP = mybir.ActivationFunctionType.Exp

B, S, H = 2, 2048, 1024
NH, DK, DV = 16, 64, 64
NCORE = 8
NCH = H // 128          # 8 contraction chunks over H
NJ = S // 512           # 4 query subtiles of 512
NKC = S // 128          # 16 key chunks
NPAIR = 2               # head pairs per core
SCALE = 1.0 / math.sqrt(DK)
NEG_BIAS = -30000.0     # exp(x + NEG_BIAS) == 0.0 in fp32 for any real score


def _r(ap):
    """Bitcast an fp32 AP to float32r so the PE runs at 1 cycle/row."""
    return ap.bitcast(F32R)


def _emit(nc, d):
    """Emit the per-core program.  d maps names -> DRAM tensor handles."""
    with tile.TileContext(nc) as tc, ExitStack() as top:
        consts = top.enter_context(tc.tile_pool(name="consts", bufs=1))
        persist = top.enter_context(tc.tile_pool(name="persist", bufs=1))

        # ---- constants / weights (resident whole kernel) ----
        wqq_sb = []
        wkk_sb = []
        for p in range(NPAIR):
            wq = consts.tile([128, NCH * 128], F32R, tag=f"wqq{p}", name=f"wqq{p}sb")
            nc.sync.dma_start(out=wq, in_=d["wqq"][p].bitcast(F32R))
            wqq_sb.append(wq)
            wk = consts.tile([128, NCH * 128], F32R, tag=f"wkk{p}", name=f"wkk{p}sb")
            nc.sync.dma_start(out=wk, in_=d["wkk"][p].bitcast(F32R))
            wkk_sb.append(wk)
        wv_sb = consts.tile([128, NCH * 256], F32R, tag="wv", name="wvsb")
        nc.sync.dma_start(out=wv_sb, in_=d["wv"][:].bitcast(F32R))
        wo_sb = consts.tile([128, 2 * 1024], F32R, tag="wo", name="wosb")
        nc.sync.dma_start(out=wo_sb, in_=d["wo"][:].bitcast(F32R))
        bq_sb = consts.tile([128, 2], F32, tag="bq", name="bqsb")
        nc.sync.dma_start(out=bq_sb, in_=d["bq"][:])
        bk_sb = consts.tile([128, 2], F32, tag="bk", name="bksb")
        nc.sync.dma_start(out=bk_sb, in_=d["bk"][:])
        bv_sb = consts.tile([128, 2], F32, tag="bv", name="bvsb")
        nc.sync.dma_start(out=bv_sb, in_=d["bv"][:])
        nbias_sb = consts.tile([128, NKC], F32, tag="nbias", name="nbiassb")
        nc.sync.dma_start(out=nbias_sb, in_=d["nbias"][:])
        mdiag_sb = consts.tile([128, 128], BF16, tag="mdiag", name="mdiagsb")
        nc.gpsimd.dma_start(out=mdiag_sb, in_=d["mdiag"][:])

        # ---- persistent activations ----
        qt_sb = []   # per pair: [128, S]; rows 0:64 head A Q^T, 64:128 head B Q^T
        kt_sb = []
        attnT = []   # per pair: [128, S]; rows = head-dim pairs, normalized attn^T
        for p in range(NPAIR):
            q = persist.tile([128, S], BF16, tag=f"qt{p}", name=f"qt{p}sb")
            qt_sb.append(q)
            k = persist.tile([128, S], BF16, tag=f"kt{p}", name=f"kt{p}sb")
            kt_sb.append(k)
            a = persist.tile([128, S], F32R, tag=f"at{p}", name=f"at{p}sb")
            attnT.append(a)
        vaug = []    # per head: [128, NKC*128]; per key-chunk [V_h(64) | ones(64)]
        for h in range(4):
            v = persist.tile([128, NKC * 128], BF16, tag=f"vaug{h}", name=f"vaug{h}sb")
            nc.vector.memset(v, 1.0)
            vaug.append(v)

        # ---- X^T (freed after the projections) ----
        with tc.tile_pool(name="xtp", bufs=1) as xtp:
            xt_sb = []
            for c in range(NCH):
                x = xtp.tile([128, S], F32R, tag=f"xt{c}", name=f"xt{c}sb")
                nc.sync.dma_start(out=x, in_=d["xt"][c * 128:(c + 1) * 128, :].bitcast(F32R))
                xt_sb.append(x)

            # ---- Q^T / K^T projections ----
            with tc.tile_pool(name="psqk", bufs=1, space="PSUM") as psqk:
                for p in range(NPAIR):
                    for wsb, bsb, dst, nm in (
                        (wqq_sb[p], bq_sb, qt_sb[p], "q"),
                        (wkk_sb[p], bk_sb, kt_sb[p], "k"),
                    ):
                        pss = [
                            psqk.tile([128, 512], F32, tag=f"ps{j}",
                                      name=f"ps{nm}{p}{j}")
                            for j in range(NJ)
                        ]
                        for c in range(NCH):
                            for j in range(NJ):
                                nc.tensor.matmul(
                                    pss[j],
                                    _r(wsb[:, c * 128:(c + 1) * 128]),
                                    _r(xt_sb[c][:, j * 512:(j + 1) * 512]),
                                    start=(c == 0), stop=(c == NCH - 1),
                                )
                        for j in range(NJ):
                            nc.vector.tensor_scalar_add(
                                dst[:, j * 512:(j + 1) * 512], pss[j],
                                bsb[:, p:p + 1],
                            )

            # ---- V projection (into V_aug halves) ----
            with tc.tile_pool(name="psv", bufs=2, space="PSUM") as psv:
                for t in range(NKC):
                    ps = psv.tile([128, 256], F32, tag="v", name=f"psv{t}")
                    for c in range(NCH):
                        nc.tensor.matmul(
                            ps,
                            _r(xt_sb[c][:, t * 128:(t + 1) * 128]),
                            _r(wv_sb[:, c * 256:(c + 1) * 256]),
                            start=(c == 0), stop=(c == NCH - 1),
                        )
                    for h in range(4):
                        nc.vector.tensor_copy(
                            vaug[h][:, t * 128:t * 128 + 64],
                            ps[:, h * 64:(h + 1) * 64],
                        )

        # ---- attention (j outer; W_O for each j's q-tiles interleaved) ----
        with (
            tc.tile_pool(name="pss", bufs=1, space="PSUM") as pssp,
            tc.tile_pool(name="pso", bufs=1, space="PSUM") as psop,
            tc.tile_pool(name="psf", bufs=2, space="PSUM") as psf,
            tc.tile_pool(name="ptp", bufs=4) as ptp,
            tc.tile_pool(name="nrm", bufs=3) as nrm,
            tc.tile_pool(name="ysb", bufs=4) as ysb,
        ):
            for j in range(NJ):
                for p in range(NPAIR):
                    hA, hB = 2 * p, 2 * p + 1
                    oA = psop.tile([128, 512], F32, tag="oA", bufs=1,
                                   name=f"oA{p}{j}")
                    oB = psop.tile([128, 512], F32, tag="oB", bufs=1,
                                   name=f"oB{p}{j}")
                    cmax = 4 * j + 3
                    for c in range(cmax + 1):
                        t = c - 4 * j
                        fo = 128 * t if t > 0 else 0
                        w = 512 - fo
                        qsl = slice(j * 512 + fo, (j + 1) * 512)
                        ksl = slice(c * 128, (c + 1) * 128)
                        sA = pssp.tile([128, 512], F32, tag="sA", bufs=2,
                                       name=f"sA{p}{j}{c}")
                        sB = pssp.tile([128, 512], F32, tag="sB", bufs=2,
                                       name=f"sB{p}{j}{c}")
                        nc.tensor.matmul(
                            sA[:, :w], kt_sb[p][0:64, ksl],
                            qt_sb[p][0:64, qsl], start=True, stop=True)
                        nc.tensor.matmul(
                            sB[:, :w], kt_sb[p][64:128, ksl],
                            qt_sb[p][64:128, qsl], start=True, stop=True)
                        pA = ptp.tile([128, 512], BF16, tag="pA", name=f"pA{p}{j}{c}")
                        pB = ptp.tile([128, 512], BF16, tag="pB", name=f"pB{p}{j}{c}")
                        nc.scalar.activation(pA[:, :w], sA[:, :w], EXP,
                                             bias=nbias_sb[:, c:c + 1], scale=SCALE)
                        nc.scalar.activation(pB[:, :w], sB[:, :w], EXP,
                                             bias=nbias_sb[:, c:c + 1], scale=SCALE)
                        if t >= 0:
                            # diagonal 128x128 block: zero keys below the diagonal
                            nc.vector.tensor_mul(pA[:, 0:128], pA[:, 0:128], mdiag_sb)
                            nc.vector.tensor_mul(pB[:, 0:128], pB[:, 0:128], mdiag_sb)
                        nc.tensor.matmul(
                            oA[:, fo:512], vaug[hA][:, ksl], pA[:, :w],
                            start=(c == 0), stop=(c == cmax))
                        nc.tensor.matmul(
                            oB[:, fo:512], vaug[hB][:, ksl], pB[:, :w],
                            start=(c == 0), stop=(c == cmax))

                    # normalize: rows 64:128 of oX hold the denominator
                    # replicated 64x (ones columns of V_aug).
                    jsl = slice(j * 512, (j + 1) * 512)
                    scrA = nrm.tile([128, 512], F32, tag="scrA", name=f"scrA{p}{j}")
                    nc.any.tensor_copy(scrA[64:128, :], oA[64:128, :])
                    recA = nrm.tile([64, 512], F32, tag="recA", name=f"recA{p}{j}")
                    nc.sync.dma_start(out=recA, in_=scrA[64:128, :])
                    nc.vector.reciprocal_approx_fast(out=recA, in_=recA)
                    nc.vector.tensor_mul(attnT[p][0:64, jsl], oA[0:64, :], recA)

                    scrB = nrm.tile([128, 512], F32, tag="scrB", name=f"scrB{p}{j}")
                    nc.any.tensor_copy(scrB[64:128, :], oB[64:128, :])
                    recB = nrm.tile([64, 512], F32, tag="recB", name=f"recB{p}{j}")
                    nc.sync.dma_start(out=recB, in_=scrB[64:128, :])
                    nc.vector.reciprocal_approx_fast(out=recB, in_=recB)
                    tmpB = nrm.tile([64, 512], F32R, tag="tmpB", name=f"tmpB{p}{j}")
                    nc.vector.tensor_mul(tmpB, oB[0:64, :], recB)
                    nc.sync.dma_start(out=attnT[p][64:128, jsl], in_=tmpB)

                    nc.vector.tensor_scalar_add(attnT[p][:, jsl], attnT[p][:, jsl],
                                                bv_sb[:, p:p + 1])

                # output projection for this j's four q-tiles (keeps PE busy
                # while the next j's exps run on ACT)
                for q in range(4 * j, 4 * j + 4):
                    for half in range(2):
                        pf = psf.tile([128, 512], F32, tag="f", name=f"pf{q}{half}")
                        for p in range(NPAIR):
                            nc.tensor.matmul(
                                pf,
                                _r(attnT[p][:, q * 128:(q + 1) * 128]),
                                _r(wo_sb[:, p * 1024 + half * 512:
                                         p * 1024 + half * 512 + 512]),
                                start=(p == 0), stop=(p == 1),
                            )
                        yt = ysb.tile([128, 512], F32, tag="y", name=f"yt{q}{half}")
                        nc.vector.tensor_copy(yt, pf)
                        nc.sync.dma_start(
                            out=d["y"][q * 128:(q + 1) * 128,
                                       half * 512:(half + 1) * 512],
                            in_=yt)

        if _DEBUG:
                        for ab, (ot, rc) in enumerate(((oA, recA), (oB, recB))):
                            dt_ = nrm.tile([128, 512], F32, tag="dbg",
                                           name=f"dbg{p}{j}{ab}")
                            nc.scalar.activation(
                                dt_, ot, mybir.ActivationFunctionType.Copy)
                            nc.sync.dma_start(out=d["dbg_o"][p, j, ab], in_=dt_)
                            nc.sync.dma_start(out=d["dbg_rec"][p, j, ab], in_=rc)

        if _DEBUG:
            for p in range(NPAIR):
                nc.sync.dma_start(out=d[f"dbg_qt{p}"][:], in_=qt_sb[p].bitcast(F32))
                nc.sync.dma_start(out=d[f"dbg_kt{p}"][:], in_=kt_sb[p].bitcast(F32))
                nc.sync.dma_start(out=d[f"dbg_at{p}"][:], in_=attnT[p].bitcast(F32))
            for h in range(4):
                nc.sync.dma_start(out=d[f"dbg_va{h}"][:], in_=vaug[h].bitcast(F32))


_NC_CACHE = {}
_DEBUG = False


def _get_nc():
    if "nc" not in _NC_CACHE:
        nc = bacc.Bacc(None, target_bir_lowering=False)
        d = {
            "xt": nc.dram_tensor("xt", [H, S], F32, kind="ExternalInput"),
            "wqq": nc.dram_tensor("wqq", [NPAIR, 128, NCH * 128], F32,
                                  kind="ExternalInput"),
            "wkk": nc.dram_tensor("wkk", [NPAIR, 128, NCH * 128], F32,
                                  kind="ExternalInput"),
            "wv": nc.dram_tensor("wv", [128, NCH * 256], F32, kind="ExternalInput"),
            "wo": nc.dram_tensor("wo", [128, 2 * 1024], F32, kind="ExternalInput"),
            "bq": nc.dram_tensor("bq", [128, 2], F32, kind="ExternalInput"),
            "bk": nc.dram_tensor("bk", [128, 2], F32, kind="ExternalInput"),
            "bv": nc.dram_tensor("bv", [128, 2], F32, kind="ExternalInput"),
            "nbias": nc.dram_tensor("nbias", [128, NKC], F32, kind="ExternalInput"),
            "mdiag": nc.dram_tensor("mdiag", [128, 128], F32, kind="ExternalInput"),
            "y": nc.dram_tensor("y", [S, H], F32, kind="ExternalOutput"),
        }
        if _DEBUG:
            for p in range(NPAIR):
                d[f"dbg_qt{p}"] = nc.dram_tensor(f"dbg_qt{p}", [128, S], F32,
                                                 kind="ExternalOutput")
                d[f"dbg_kt{p}"] = nc.dram_tensor(f"dbg_kt{p}", [128, S], F32,
                                                 kind="ExternalOutput")
                d[f"dbg_at{p}"] = nc.dram_tensor(f"dbg_at{p}", [128, S], F32,
                                                 kind="ExternalOutput")
            for h in range(4):
                d[f"dbg_va{h}"] = nc.dram_tensor(f"dbg_va{h}", [128, NKC * 128],
                                                 F32, kind="ExternalOutput")
            d["dbg_o"] = nc.dram_tensor("dbg_o", [NPAIR, NJ, 2, 128, 512], F32,
                                        kind="ExternalOutput")
            d["dbg_rec"] = nc.dram_tensor("dbg_rec", [NPAIR, NJ, 2, 64, 512], F32,
                                          kind="ExternalOutput")
        _emit(nc, d)
        nc.finalize()
        _NC_CACHE["nc"] = nc
    return _NC_CACHE["nc"]


def _chunked(w, ncols):
    """[H, ncols] -> [128, NCH*ncols] with chunk c of rows at cols c*ncols."""
    return np.ascontiguousarray(
        w.reshape(NCH, 128, ncols).transpose(1, 0, 2).reshape(128, NCH * ncols))


def _make_in_maps(batch, input_ids, W_Q, W_K, W_V, b_Q, b_K, b_V, W_O):
    mdiag = np.triu(np.ones((128, 128), np.float32))
    in_maps = []
    for core in range(NCORE):
        b, g = divmod(core, 4)
        base = 256 * g  # first feature column of this core's 4 heads
        wqq = np.stack([_chunked(W_Q[:, base + 128 * p: base + 128 * (p + 1)], 128)
                        for p in range(NPAIR)])
        wkk = np.stack([_chunked(W_K[:, base + 128 * p: base + 128 * (p + 1)], 128)
                        for p in range(NPAIR)])
        wv = _chunked(W_V[:, base: base + 256], 256)
        wo = np.ascontiguousarray(
            W_O[base: base + 256, :].reshape(2, 128, H)
            .transpose(1, 0, 2).reshape(128, 2 * H))
        bq = np.stack([b_Q[base + 128 * p: base + 128 * (p + 1)]
                       for p in range(NPAIR)], axis=1)
        bk = np.stack([b_K[base + 128 * p: base + 128 * (p + 1)]
                       for p in range(NPAIR)], axis=1)
        bv = np.stack([b_V[base + 128 * p: base + 128 * (p + 1)]
                       for p in range(NPAIR)], axis=1)
        keep = input_ids[b] != 0
        nbias = np.where(keep, 0.0, NEG_BIAS).astype(np.float32)
        nbias = np.ascontiguousarray(nbias.reshape(NKC, 128).T)
        xt = np.ascontiguousarray(batch[b].T)
        in_maps.append({
            "xt": xt, "wqq": wqq, "wkk": wkk, "wv": wv, "wo": wo,
            "bq": np.ascontiguousarray(bq), "bk": np.ascontiguousarray(bk),
            "bv": np.ascontiguousarray(bv), "nbias": nbias, "mdiag": mdiag,
        })
    return in_maps


def _run(in_maps, **kwargs):
    nc = _get_nc()
    return run_bass_kernel_spmd(nc, in_maps, core_ids=list(range(NCORE)), **kwargs)


def kernel(batch, input_ids, W_Q, W_K, W_V, b_Q, b_K, b_V, W_O, b_O,
           _results_out=None, **run_kwargs):
    batch = np.asarray(batch, np.float32)
    input_ids = np.asarray(input_ids)
    W_Q, W_K, W_V = (np.asarray(a, np.float32) for a in (W_Q, W_K, W_V))
    b_Q, b_K, b_V = (np.asarray(a, np.float32) for a in (b_Q, b_K, b_V))
    W_O = np.asarray(W_O, np.float32)
    b_O = np.asarray(b_O, np.float32)

    in_maps = _make_in_maps(batch, input_ids, W_Q, W_K, W_V, b_Q, b_K, b_V, W_O)
    res = _run(in_maps, **run_kwargs)
    if _results_out is not None:
        _results_out.append(res)
    ys = [res.results[c]["y"] for c in range(NCORE)]
    out = np.stack([sum(ys[4 * b: 4 * b + 4]) for b in range(B)], axis=0)
    return (out + b_O).astype(np.float32)



# revision 3
# speedup vs baseline: 1.1044x; 1.1044x over previous
"""Multi-head attention (B=2, S=2048, H=1024, NH=16, DK=DV=64) on 8 TRN2 cores.

Sharding: data-parallel over batch (2 groups of 4 cores) x tensor-parallel
over heads (4 heads per core).  Each core computes, for its batch sample and
its 4 heads:
    Q^T/K^T projections (features on partitions), V projection (natural),
    S^T = K @ Q^T per 128-key chunk (causal chunks only; the two heads of a
    pair run as concurrent row-tiled K=64 matmuls into one 2-bank PSUM tile),
    P^T = exp(S^T/8 + pad_bias)  (one ACTIVATE covers both heads),
    out^T = V_aug^T @ P^T  where V_aug = [V | ones] for even heads and
    [ones | V] for odd heads, so values and 1/denominator stay
    partition-aligned for both halves of attn^T,
    y_partial = attn^T.T @ W_O_rows   (row-sharded W_O).
Host sums the 4 bf16 partials per batch and adds b_V @ W_O + b_O (exact
fold of the V bias through the output projection).

Everything on the wide data path is bf16 (fp32 accumulation in PSUM):
halves HBM traffic and LDWEIGHTS time vs fp32.  Engine balance: PE does
matmuls only; ACT does projection writebacks (idle phase) + all exps;
GpSimd does the causal-diagonal masks and half the y copies; DVE does
normalization and the other half of the y copies.
"""

import math
from contextlib import ExitStack

import numpy as np

import concourse.bass as bass
import concourse.mybir as mybir
from concourse import bacc
import concourse.tile as tile
from concourse.bass_utils import run_bass_kernel_spmd

F32 = mybir.dt.float32
BF16 = mybir.dt.bfloat16
EXP = mybir.ActivationFunctionType.Exp
IDENT = mybir.ActivationFunctionType.Identity
COPY = mybir.ActivationFunctionType.Copy

B, S, H = 2, 2048, 1024
NH, DK, DV = 16, 64, 64
NCORE = 8
NCH = H // 128          # 8 contraction chunks over H
NJ = S // 512           # 4 query subtiles of 512
NKC = S // 128          # 16 key chunks
NPAIR = 2               # head pairs per core
SCALE = 1.0 / math.sqrt(DK)
NEG_BIAS = -30000.0     # exp(x + NEG_BIAS) == 0.0 in fp32 for any real score


def _emit(nc, d):
    with tile.TileContext(nc) as tc, ExitStack() as top:
        consts = top.enter_context(tc.tile_pool(name="consts", bufs=1))
        persist = top.enter_context(tc.tile_pool(name="persist", bufs=1))

        # ---- persistent activations ----
        qt_sb = []   # per pair: [128, S] bf16; rows 0:64 head A, 64:128 head B
        kt_sb = []
        attnT = []   # per pair: [128, S] bf16 normalized attn^T
        for p in range(NPAIR):
            qt_sb.append(persist.tile([128, S], BF16, tag=f"qt{p}", name=f"qt{p}sb"))
            kt_sb.append(persist.tile([128, S], BF16, tag=f"kt{p}", name=f"kt{p}sb"))
            attnT.append(persist.tile([128, S], BF16, tag=f"at{p}", name=f"at{p}sb"))
        # V_aug per head: [128 keys, NKC*128]; chunk t block is [V|ones] for
        # even heads, [ones|V] for odd heads.
        vaug = []
        for h in range(4):
            v = persist.tile([128, NKC * 128], BF16, tag=f"vaug{h}", name=f"vaug{h}sb")
            nc.vector.memset(v, 1.0)
            vaug.append(v)

        # ---- X^T (freed after the projections) ----
        with tc.tile_pool(name="xtp", bufs=1) as xtp:
            xt_sb = [xtp.tile([128, S], BF16, tag=f"xt{c}", name=f"xt{c}sb")
                     for c in range(NCH)]

            # DMA order: xt chunk 0 first so the first matmuls start ~2us in,
            # then pair-0 weights, then the rest of xt, then later weights.
            nc.sync.dma_start(out=xt_sb[0], in_=d["xt"][0:128, :])
            wqq_sb = []
            wkk_sb = []
            for p in range(NPAIR):
                wqq_sb.append(consts.tile([128, NCH * 128], BF16, tag=f"wqq{p}",
                                          name=f"wqq{p}sb"))
                wkk_sb.append(consts.tile([128, NCH * 128], BF16, tag=f"wkk{p}",
                                          name=f"wkk{p}sb"))
            nc.sync.dma_start(out=wqq_sb[0], in_=d["wqq"][0])
            nc.sync.dma_start(out=wkk_sb[0], in_=d["wkk"][0])
            for c in range(1, NCH):
                nc.sync.dma_start(out=xt_sb[c], in_=d["xt"][c * 128:(c + 1) * 128, :])
            nc.sync.dma_start(out=wqq_sb[1], in_=d["wqq"][1])
            nc.sync.dma_start(out=wkk_sb[1], in_=d["wkk"][1])
            wv_sb = consts.tile([128, NCH * 256], BF16, tag="wv", name="wvsb")
            nc.sync.dma_start(out=wv_sb, in_=d["wv"][:])
            wo_sb = consts.tile([128, 2 * 1024], BF16, tag="wo", name="wosb")
            nc.sync.dma_start(out=wo_sb, in_=d["wo"][:])
            bq_sb = consts.tile([128, 2], F32, tag="bq", name="bqsb")
            nc.sync.dma_start(out=bq_sb, in_=d["bq"][:])
            bk_sb = consts.tile([128, 2], F32, tag="bk", name="bksb")
            nc.sync.dma_start(out=bk_sb, in_=d["bk"][:])
            nbias_sb = consts.tile([128, NKC], F32, tag="nbias", name="nbiassb")
            nc.sync.dma_start(out=nbias_sb, in_=d["nbias"][:])
            mdiag_sb = consts.tile([128, 2, 128], BF16, tag="mdiag", name="mdiagsb")
            nc.gpsimd.dma_start(out=mdiag_sb, in_=d["mdiag"][:])

            # ---- Q^T / K^T projections (writeback + bias on ACT) ----
            with tc.tile_pool(name="psqk", bufs=1, space="PSUM") as psqk:
                for p in range(NPAIR):
                    for wsb, bsb, dst, nm in (
                        (wqq_sb[p], bq_sb, qt_sb[p], "q"),
                        (wkk_sb[p], bk_sb, kt_sb[p], "k"),
                    ):
                        pss = [
                            psqk.tile([128, 512], F32, tag=f"ps{j}",
                                      name=f"ps{nm}{p}{j}")
                            for j in range(NJ)
                        ]
                        for c in range(NCH):
                            for j in range(NJ):
                                nc.tensor.matmul(
                                    pss[j],
                                    wsb[:, c * 128:(c + 1) * 128],
                                    xt_sb[c][:, j * 512:(j + 1) * 512],
                                    start=(c == 0), stop=(c == NCH - 1),
                                )
                        for j in range(NJ):
                            nc.scalar.activation(
                                dst[:, j * 512:(j + 1) * 512], pss[j],
                                IDENT, bias=bsb[:, p:p + 1],
                            )

            # ---- V projection (into V_aug; writeback on ACT) ----
            with tc.tile_pool(name="psv", bufs=2, space="PSUM") as psv:
                for t in range(NKC):
                    ps = psv.tile([128, 256], F32, tag="v", name=f"psv{t}")
                    for c in range(NCH):
                        nc.tensor.matmul(
                            ps,
                            xt_sb[c][:, t * 128:(t + 1) * 128],
                            wv_sb[:, c * 256:(c + 1) * 256],
                            start=(c == 0), stop=(c == NCH - 1),
                        )
                    for h in range(4):
                        off = t * 128 + (0 if h % 2 == 0 else 64)
                        nc.scalar.activation(
                            vaug[h][:, off:off + 64],
                            ps[:, h * 64:(h + 1) * 64], COPY,
                        )

        # ---- attention (W_O for j-1 interleaved after att(j)) ----
        with (
            tc.tile_pool(name="psab", bufs=2, space="PSUM") as psab,
            tc.tile_pool(name="pso", bufs=1, space="PSUM") as psop,
            tc.tile_pool(name="psf", bufs=2, space="PSUM") as psf,
            tc.tile_pool(name="ptp", bufs=3) as ptp,
            tc.tile_pool(name="nrm", bufs=2) as nrm,
            tc.tile_pool(name="ysb", bufs=4) as ysb,
        ):
            def emit_wo(j):
                # output projection for j's four q-tiles
                for q in range(4 * j, 4 * j + 4):
                    for half in range(2):
                        pf = psf.tile([128, 512], F32, tag="f", name=f"pf{q}{half}")
                        for p in range(NPAIR):
                            nc.tensor.matmul(
                                pf,
                                attnT[p][:, q * 128:(q + 1) * 128],
                                wo_sb[:, p * 1024 + half * 512:
                                      p * 1024 + half * 512 + 512],
                                start=(p == 0), stop=(p == 1),
                            )
                        yt = ysb.tile([128, 512], BF16, tag="y", name=f"yt{q}{half}")
                        nc.vector.tensor_copy(yt, pf)
                        nc.sync.dma_start(
                            out=d["y"][q * 128:(q + 1) * 128,
                                       half * 512:(half + 1) * 512],
                            in_=yt)

            for j in range(NJ):
                for p in range(NPAIR):
                    hA, hB = 2 * p, 2 * p + 1
                    oA = psop.tile([128, 512], F32, tag="oA", bufs=1,
                                   name=f"oA{p}{j}")
                    oB = psop.tile([128, 512], F32, tag="oB", bufs=1,
                                   name=f"oB{p}{j}")
                    cmax = 4 * j + 3
                    for c in range(cmax + 1):
                        t = c - 4 * j
                        fo = 128 * t if t > 0 else 0
                        w = 512 - fo
                        qsl = slice(j * 512 + fo, (j + 1) * 512)
                        ksl = slice(c * 128, (c + 1) * 128)
                        sAB = psab.tile([128, 2, 512], F32, tag="sAB",
                                        name=f"sAB{p}{j}{c}")
                        nc.tensor.matmul(
                            sAB[:, 0, :w], kt_sb[p][0:64, ksl],
                            qt_sb[p][0:64, qsl], start=True, stop=True)
                        nc.tensor.matmul(
                            sAB[:, 1, :w], kt_sb[p][64:128, ksl],
                            qt_sb[p][64:128, qsl], start=True, stop=True)
                        pAB = ptp.tile([128, 2, 512], BF16, tag="pAB",
                                       name=f"pAB{p}{j}{c}")
                        nc.scalar.activation(pAB[:, :, :w], sAB[:, :, :w], EXP,
                                             bias=nbias_sb[:, c:c + 1],
                                             scale=SCALE)
                        if t >= 0:
                            # diagonal 128x128 block: zero keys below diagonal
                            nc.gpsimd.tensor_mul(pAB[:, :, 0:128],
                                                 pAB[:, :, 0:128], mdiag_sb)
                        nc.tensor.matmul(
                            oA[:, fo:512], vaug[hA][:, ksl], pAB[:, 0, :w],
                            start=(c == 0), stop=(c == cmax))
                        nc.tensor.matmul(
                            oB[:, fo:512], vaug[hB][:, ksl], pAB[:, 1, :w],
                            start=(c == 0), stop=(c == cmax))

                    # normalize: head A denom at oA[64:128], head B denom at
                    # oB[0:64] (ones-half layouts differ so values/recip end
                    # up partition-aligned after one cross DMA each).
                    jsl = slice(j * 512, (j + 1) * 512)
                    scr = nrm.tile([128, 512], F32, tag="scr", name=f"scr{p}{j}")
                    nc.vector.tensor_copy(scr[64:128, :], oA[64:128, :])
                    nc.vector.tensor_copy(scr[0:64, :], oB[0:64, :])
                    rec = nrm.tile([128, 512], F32, tag="rec", name=f"rec{p}{j}")
                    nc.sync.dma_start(out=rec[0:64, :], in_=scr[64:128, :])
                    nc.sync.dma_start(out=rec[64:128, :], in_=scr[0:64, :])
                    nc.vector.reciprocal_approx_fast(out=rec, in_=rec)
                    nc.vector.tensor_mul(attnT[p][0:64, jsl], oA[0:64, :],
                                         rec[0:64, :])
                    nc.vector.tensor_mul(attnT[p][64:128, jsl], oB[64:128, :],
                                         rec[64:128, :])

                if j > 0:
                    emit_wo(j - 1)
            emit_wo(NJ - 1)


_NC_CACHE = {}


def _get_nc():
    if "nc" not in _NC_CACHE:
        nc = bacc.Bacc(None, target_bir_lowering=False)
        d = {
            "xt": nc.dram_tensor("xt", [H, S], BF16, kind="ExternalInput"),
            "wqq": nc.dram_tensor("wqq", [NPAIR, 128, NCH * 128], BF16,
                                  kind="ExternalInput"),
            "wkk": nc.dram_tensor("wkk", [NPAIR, 128, NCH * 128], BF16,
                                  kind="ExternalInput"),
            "wv": nc.dram_tensor("wv", [128, NCH * 256], BF16,
                                 kind="ExternalInput"),
            "wo": nc.dram_tensor("wo", [128, 2 * 1024], BF16,
                                 kind="ExternalInput"),
            "bq": nc.dram_tensor("bq", [128, 2], F32, kind="ExternalInput"),
            "bk": nc.dram_tensor("bk", [128, 2], F32, kind="ExternalInput"),
            "nbias": nc.dram_tensor("nbias", [128, NKC], F32,
                                    kind="ExternalInput"),
            "mdiag": nc.dram_tensor("mdiag", [128, 2, 128], BF16,
                                    kind="ExternalInput"),
            "y": nc.dram_tensor("y", [S, H], BF16, kind="ExternalOutput"),
        }
        _emit(nc, d)
        nc.finalize()
        _NC_CACHE["nc"] = nc
    return _NC_CACHE["nc"]


def _bf16(a):
    import ml_dtypes
    return np.ascontiguousarray(a.astype(ml_dtypes.bfloat16))


def _chunked(w, ncols):
    """[H, ncols] -> [128, NCH*ncols] with chunk c of rows at cols c*ncols."""
    return np.ascontiguousarray(
        w.reshape(NCH, 128, ncols).transpose(1, 0, 2).reshape(128, NCH * ncols))


def _make_in_maps(batch, input_ids, W_Q, W_K, W_V, b_Q, b_K, W_O):
    mdiag = np.broadcast_to(np.triu(np.ones((128, 128), np.float32)),
                            (2, 128, 128)).transpose(1, 0, 2)
    mdiag = _bf16(np.ascontiguousarray(mdiag))
    in_maps = []
    for core in range(NCORE):
        b, g = divmod(core, 4)
        base = 256 * g  # first feature column of this core's 4 heads
        wqq = np.stack([_chunked(W_Q[:, base + 128 * p: base + 128 * (p + 1)], 128)
                        for p in range(NPAIR)])
        wkk = np.stack([_chunked(W_K[:, base + 128 * p: base + 128 * (p + 1)], 128)
                        for p in range(NPAIR)])
        wv = _chunked(W_V[:, base: base + 256], 256)
        wo = np.ascontiguousarray(
            W_O[base: base + 256, :].reshape(2, 128, H)
            .transpose(1, 0, 2).reshape(128, 2 * H))
        bq = np.stack([b_Q[base + 128 * p: base + 128 * (p + 1)]
                       for p in range(NPAIR)], axis=1)
        bk = np.stack([b_K[base + 128 * p: base + 128 * (p + 1)]
                       for p in range(NPAIR)], axis=1)
        keep = input_ids[b] != 0
        nbias = np.where(keep, 0.0, NEG_BIAS).astype(np.float32)
        nbias = np.ascontiguousarray(nbias.reshape(NKC, 128).T)
        xt = np.ascontiguousarray(batch[b].T)
        in_maps.append({
            "xt": _bf16(xt), "wqq": _bf16(wqq), "wkk": _bf16(wkk),
            "wv": _bf16(wv), "wo": _bf16(wo),
            "bq": np.ascontiguousarray(bq), "bk": np.ascontiguousarray(bk),
            "nbias": nbias, "mdiag": mdiag,
        })
    return in_maps


def _run(in_maps, **kwargs):
    nc = _get_nc()
    return run_bass_kernel_spmd(nc, in_maps, core_ids=list(range(NCORE)), **kwargs)


def kernel(batch, input_ids, W_Q, W_K, W_V, b_Q, b_K, b_V, W_O, b_O,
           _results_out=None, **run_kwargs):
    batch = np.asarray(batch, np.float32)
    input_ids = np.asarray(input_ids)
    W_Q, W_K, W_V = (np.asarray(a, np.float32) for a in (W_Q, W_K, W_V))
    b_Q, b_K, b_V = (np.asarray(a, np.float32) for a in (b_Q, b_K, b_V))
    W_O = np.asarray(W_O, np.float32)
    b_O = np.asarray(b_O, np.float32)

    in_maps = _make_in_maps(batch, input_ids, W_Q, W_K, W_V, b_Q, b_K, W_O)
    res = _run(in_maps, **run_kwargs)
    if _results_out is not None:
        _results_out.append(res)
    ys = [np.asarray(res.results[c]["y"], np.float32) for c in range(NCORE)]
    out = np.stack([sum(ys[4 * b: 4 * b + 4]) for b in range(B)], axis=0)
    # exact fold: attn rows sum to 1, so the V bias passes through W_O
    bias = b_V @ W_O + b_O
    return (out + bias).astype(np.float32)


# revision 5
# speedup vs baseline: 1.3096x; 1.1858x over previous
"""Multi-head attention (B=2, S=2048, H=1024, NH=16, DK=DV=64) on 8 TRN2 cores.

Sharding: data-parallel over batch (2 groups of 4 cores) x tensor-parallel
over heads (4 heads per core).  Each core computes, for its batch sample and
its 4 heads:
    Q^T/K^T projections (features on partitions), V projection (natural),
    S^T = K @ Q^T per 128-key chunk (causal chunks only; the two heads of a
    pair run as concurrent row-tiled K=64 matmuls into one 2-bank PSUM tile),
    P^T = exp(S^T/8 + pad_bias)  (one ACTIVATE covers both heads),
    out^T = V_aug^T @ P^T  where V_aug = [V | ones] for even heads and
    [ones | V] for odd heads, so values and 1/denominator stay
    partition-aligned for both halves of attn^T,
    y_partial = attn^T.T @ W_O_rows   (row-sharded W_O).
Host sums the 4 bf16 partials per batch and adds b_V @ W_O + b_O (exact
fold of the V bias through the output projection).

The emission is hand-staged so the ACT engine (exp is the serial bottleneck,
~88us/core) starts ~12us in and never starves, while projection/output
matmuls fill the PE between attention chunks and keep the PE HAM-warm:

    S1   : pair-0 Q/K projections as an 8-bank PSUM wave (c-outer), paced by
           the x^T DMA stream
    S2-5 : per j: V-projection chunks for j's keys, then att(j, pair0)
    S6-7 : pair-1 Q/K projection groups woven between att(0..1, pair1)
    S8-9 : att(2..3, pair1) with W_O(0..2) woven into the chunk loops
    S10  : W_O(3)

Everything on the wide data path is bf16 (fp32 accumulation in PSUM).
Engine balance: PE matmuls only; ACT exps only; DVE does projection
writebacks, normalization, and y casts; GpSimd does the causal-diagonal
masks.  All PSUM pool scopes are arranged to stay within the 8 banks.
"""

import math
from contextlib import ExitStack

import numpy as np

import concourse.bass as bass
import concourse.mybir as mybir
from concourse import bacc
import concourse.tile as tile
from concourse.bass_utils import run_bass_kernel_spmd

F32 = mybir.dt.float32
BF16 = mybir.dt.bfloat16
EXP = mybir.ActivationFunctionType.Exp

B, S, H = 2, 2048, 1024
NH, DK, DV = 16, 64, 64
NCORE = 8
NCH = H // 128          # 8 contraction chunks over H
NJ = S // 512           # 4 query subtiles of 512
NKC = S // 128          # 16 key chunks
NPAIR = 2               # head pairs per core
SCALE = 1.0 / math.sqrt(DK)
NEG_BIAS = -30000.0     # exp(x + NEG_BIAS) == 0.0 in fp32 for any real score


def _emit(nc, d):
    with tile.TileContext(nc) as tc, ExitStack() as top:
        consts = top.enter_context(tc.tile_pool(name="consts", bufs=1))
        persist = top.enter_context(tc.tile_pool(name="persist", bufs=1))
        xtp = top.enter_context(tc.tile_pool(name="xtp", bufs=1))

        # ---- persistent activations ----
        qt_sb = []   # per pair: [128, S] bf16; rows 0:64 head A, 64:128 head B
        kt_sb = []
        attnT = []   # per pair: [128, S] bf16 normalized attn^T
        for p in range(NPAIR):
            qt_sb.append(persist.tile([128, S], BF16, tag=f"qt{p}", name=f"qt{p}sb"))
            kt_sb.append(persist.tile([128, S], BF16, tag=f"kt{p}", name=f"kt{p}sb"))
            attnT.append(persist.tile([128, S], BF16, tag=f"at{p}", name=f"at{p}sb"))
        # V_aug per head: [128 keys, NKC*128]; chunk t block is [V|ones] for
        # even heads, [ones|V] for odd heads.
        vaug = []
        for h in range(4):
            v = persist.tile([128, NKC * 128], BF16, tag=f"vaug{h}", name=f"vaug{h}sb")
            nc.vector.memset(v, 1.0)
            vaug.append(v)

        xt_sb = [xtp.tile([128, S], BF16, tag=f"xt{c}", name=f"xt{c}sb")
                 for c in range(NCH)]

        # DMA order = consumption order: pair-0 weights + small consts, the
        # x^T stream, then later-phase weights.
        wqq_sb = []
        wkk_sb = []
        for p in range(NPAIR):
            wqq_sb.append(consts.tile([128, NCH * 128], BF16, tag=f"wqq{p}",
                                      name=f"wqq{p}sb"))
            wkk_sb.append(consts.tile([128, NCH * 128], BF16, tag=f"wkk{p}",
                                      name=f"wkk{p}sb"))
        nc.sync.dma_start(out=wqq_sb[0], in_=d["wqq"][0])
        nc.sync.dma_start(out=wkk_sb[0], in_=d["wkk"][0])
        bq_sb = consts.tile([128, 2], F32, tag="bq", name="bqsb")
        nc.sync.dma_start(out=bq_sb, in_=d["bq"][:])
        bk_sb = consts.tile([128, 2], F32, tag="bk", name="bksb")
        nc.sync.dma_start(out=bk_sb, in_=d["bk"][:])
        nbias_sb = consts.tile([128, NKC], F32, tag="nbias", name="nbiassb")
        nc.sync.dma_start(out=nbias_sb, in_=d["nbias"][:])
        for c in range(NCH):
            nc.sync.dma_start(out=xt_sb[c], in_=d["xt"][c * 128:(c + 1) * 128, :])
        wv_sb = consts.tile([128, NCH * 256], BF16, tag="wv", name="wvsb")
        nc.sync.dma_start(out=wv_sb, in_=d["wv"][:])
        nc.sync.dma_start(out=wqq_sb[1], in_=d["wqq"][1])
        nc.sync.dma_start(out=wkk_sb[1], in_=d["wkk"][1])
        wo_sb = consts.tile([128, 2 * 1024], BF16, tag="wo", name="wosb")
        nc.sync.dma_start(out=wo_sb, in_=d["wo"][:])
        mdiag_sb = consts.tile([128, 2, 128], BF16, tag="mdiag", name="mdiagsb")
        nc.gpsimd.dma_start(out=mdiag_sb, in_=d["mdiag"][:])

        # ---- S1: pair-0 Q/K projections, 8-bank wave paced by the xt DMA ----
        with tc.tile_pool(name="psqk8", bufs=1, space="PSUM") as psqk8:
            pss = {}
            for qk in range(2):
                for j in range(NJ):
                    pss[qk, j] = psqk8.tile([128, 512], F32, tag=f"pp{qk}{j}",
                                            name=f"pp{qk}{j}")
            for c in range(NCH):
                for qk, wsb in ((0, wqq_sb[0]), (1, wkk_sb[0])):
                    for j in range(NJ):
                        nc.tensor.matmul(
                            pss[qk, j],
                            wsb[:, c * 128:(c + 1) * 128],
                            xt_sb[c][:, j * 512:(j + 1) * 512],
                            start=(c == 0), stop=(c == NCH - 1),
                        )
            for qk, (bsb, dst) in enumerate(((bq_sb, qt_sb[0]), (bk_sb, kt_sb[0]))):
                for j in range(NJ):
                    nc.vector.tensor_scalar_add(
                        dst[:, j * 512:(j + 1) * 512], pss[qk, j], bsb[:, 0:1])

        with (
            tc.tile_pool(name="psab", bufs=2, space="PSUM") as psab,
            tc.tile_pool(name="pso", bufs=1, space="PSUM") as psop,
            tc.tile_pool(name="ptp", bufs=3) as ptp,
            tc.tile_pool(name="nrm", bufs=2) as nrm,
            tc.tile_pool(name="ysb", bufs=4) as ysb,
        ):
            def qkproj_group(pool, p, qk, j):
                """Pair-1 style single-tile projection group (c-inner)."""
                wsb = (wqq_sb, wkk_sb)[qk][p]
                bsb = (bq_sb, bk_sb)[qk]
                dst = (qt_sb, kt_sb)[qk][p]
                ps = pool.tile([128, 512], F32, tag="qk", name=f"qk{p}{qk}{j}")
                for c in range(NCH):
                    nc.tensor.matmul(
                        ps, wsb[:, c * 128:(c + 1) * 128],
                        xt_sb[c][:, j * 512:(j + 1) * 512],
                        start=(c == 0), stop=(c == NCH - 1),
                    )
                nc.vector.tensor_scalar_add(
                    dst[:, j * 512:(j + 1) * 512], ps, bsb[:, p:p + 1])

            def vproj(pool, t):
                ps = pool.tile([128, 256], F32, tag="v", name=f"psv{t}")
                for c in range(NCH):
                    nc.tensor.matmul(
                        ps, xt_sb[c][:, t * 128:(t + 1) * 128],
                        wv_sb[:, c * 256:(c + 1) * 256],
                        start=(c == 0), stop=(c == NCH - 1),
                    )
                for h in range(4):
                    off = t * 128 + (0 if h % 2 == 0 else 64)
                    nc.vector.tensor_copy(vaug[h][:, off:off + 64],
                                          ps[:, h * 64:(h + 1) * 64])

            def att(j, p, weave=None):
                """Attention for (j, pair p): scores/exp/mask/PV + normalize.
                weave: optional {chunk_index: fn} emitted after that chunk."""
                hA, hB = 2 * p, 2 * p + 1
                oA = psop.tile([128, 512], F32, tag="oA", bufs=1, name=f"oA{p}{j}")
                oB = psop.tile([128, 512], F32, tag="oB", bufs=1, name=f"oB{p}{j}")
                cmax = 4 * j + 3
                for c in range(cmax + 1):
                    t = c - 4 * j
                    fo = 128 * t if t > 0 else 0
                    w = 512 - fo
                    qsl = slice(j * 512 + fo, (j + 1) * 512)
                    ksl = slice(c * 128, (c + 1) * 128)
                    sAB = psab.tile([128, 2, 512], F32, tag="sAB",
                                    name=f"sAB{p}{j}{c}")
                    nc.tensor.matmul(
                        sAB[:, 0, :w], kt_sb[p][0:64, ksl],
                        qt_sb[p][0:64, qsl], start=True, stop=True)
                    nc.tensor.matmul(
                        sAB[:, 1, :w], kt_sb[p][64:128, ksl],
                        qt_sb[p][64:128, qsl], start=True, stop=True)
                    pAB = ptp.tile([128, 2, 512], BF16, tag="pAB",
                                   name=f"pAB{p}{j}{c}")
                    if w == 512:
                        nc.scalar.activation(pAB[:, :, :], sAB[:, :, :], EXP,
                                             bias=nbias_sb[:, c:c + 1],
                                             scale=SCALE)
                    else:
                        nc.scalar.activation(pAB[:, :, :w], sAB[:, :, :w], EXP,
                                             bias=nbias_sb[:, c:c + 1],
                                             scale=SCALE)
                    if t >= 0:
                        # diagonal 128x128 block: zero keys below the diagonal
                        nc.gpsimd.tensor_mul(pAB[:, :, 0:128],
                                             pAB[:, :, 0:128], mdiag_sb)
                    nc.tensor.matmul(
                        oA[:, fo:512], vaug[hA][:, ksl], pAB[:, 0, :w],
                        start=(c == 0), stop=(c == cmax))
                    nc.tensor.matmul(
                        oB[:, fo:512], vaug[hB][:, ksl], pAB[:, 1, :w],
                        start=(c == 0), stop=(c == cmax))
                    if weave and c in weave:
                        weave[c]()

                # normalize: head A denom at oA[64:128], head B denom at
                # oB[0:64]; one cross DMA each makes values/recip lane-aligned.
                jsl = slice(j * 512, (j + 1) * 512)
                scr = nrm.tile([128, 512], F32, tag="scr", name=f"scr{p}{j}")
                nc.vector.tensor_copy(scr[64:128, :], oA[64:128, :])
                nc.vector.tensor_copy(scr[0:64, :], oB[0:64, :])
                rec = nrm.tile([128, 512], F32, tag="rec", name=f"rec{p}{j}")
                nc.sync.dma_start(out=rec[0:64, :], in_=scr[64:128, :])
                nc.sync.dma_start(out=rec[64:128, :], in_=scr[0:64, :])
                nc.vector.reciprocal_approx_fast(out=rec, in_=rec)
                nc.vector.tensor_mul(attnT[p][0:64, jsl], oA[0:64, :],
                                     rec[0:64, :])
                nc.vector.tensor_mul(attnT[p][64:128, jsl], oB[64:128, :],
                                     rec[64:128, :])

            def wo_tile(pool, q, half):
                pf = pool.tile([128, 512], F32, tag="f", name=f"pf{q}{half}")
                for p in range(NPAIR):
                    nc.tensor.matmul(
                        pf, attnT[p][:, q * 128:(q + 1) * 128],
                        wo_sb[:, p * 1024 + half * 512:
                              p * 1024 + half * 512 + 512],
                        start=(p == 0), stop=(p == 1),
                    )
                yt = ysb.tile([128, 512], BF16, tag="y", name=f"yt{q}{half}")
                nc.vector.tensor_copy(yt, pf)
                nc.sync.dma_start(
                    out=d["y"][q * 128:(q + 1) * 128,
                               half * 512:(half + 1) * 512],
                    in_=yt)

            # ---- S2-S5: V projection woven with att(*, pair0) ----
            with tc.tile_pool(name="psv", bufs=2, space="PSUM") as psv:
                for j in range(NJ):
                    for t in range(4 * j, 4 * j + 4):
                        vproj(psv, t)
                    att(j, 0)

            # ---- S6-S7: pair-1 projections woven with att(0..1, pair1) ----
            with tc.tile_pool(name="psqk2", bufs=2, space="PSUM") as psqk2:
                qkproj_group(psqk2, 1, 0, 0)
                qkproj_group(psqk2, 1, 1, 0)
                att(0, 1)
                qkproj_group(psqk2, 1, 0, 1)
                qkproj_group(psqk2, 1, 1, 1)
                att(1, 1)
                qkproj_group(psqk2, 1, 0, 2)
                qkproj_group(psqk2, 1, 1, 2)
                qkproj_group(psqk2, 1, 0, 3)
                qkproj_group(psqk2, 1, 1, 3)

            # ---- S8-S10: att(2..3, pair1) with W_O woven in ----
            with tc.tile_pool(name="psf", bufs=2, space="PSUM") as psf:
                def wo_j(j):
                    def fn():
                        for q in range(4 * j, 4 * j + 4):
                            for half in range(2):
                                wo_tile(psf, q, half)
                    return fn

                att(2, 1, weave={3: wo_j(0), 8: wo_j(1)})
                att(3, 1, weave={5: wo_j(2)})
                wo_j(3)()


_NC_CACHE = {}


def _get_nc():
    if "nc" not in _NC_CACHE:
        nc = bacc.Bacc(None, target_bir_lowering=False)
        d = {
            "xt": nc.dram_tensor("xt", [H, S], BF16, kind="ExternalInput"),
            "wqq": nc.dram_tensor("wqq", [NPAIR, 128, NCH * 128], BF16,
                                  kind="ExternalInput"),
            "wkk": nc.dram_tensor("wkk", [NPAIR, 128, NCH * 128], BF16,
                                  kind="ExternalInput"),
            "wv": nc.dram_tensor("wv", [128, NCH * 256], BF16,
                                 kind="ExternalInput"),
            "wo": nc.dram_tensor("wo", [128, 2 * 1024], BF16,
                                 kind="ExternalInput"),
            "bq": nc.dram_tensor("bq", [128, 2], F32, kind="ExternalInput"),
            "bk": nc.dram_tensor("bk", [128, 2], F32, kind="ExternalInput"),
            "nbias": nc.dram_tensor("nbias", [128, NKC], F32,
                                    kind="ExternalInput"),
            "mdiag": nc.dram_tensor("mdiag", [128, 2, 128], BF16,
                                    kind="ExternalInput"),
            "y": nc.dram_tensor("y", [S, H], BF16, kind="ExternalOutput"),
        }
        _emit(nc, d)
        nc.finalize()
        _NC_CACHE["nc"] = nc
    return _NC_CACHE["nc"]


def _bf16(a):
    import ml_dtypes
    return np.ascontiguousarray(a.astype(ml_dtypes.bfloat16))


def _chunked(w, ncols):
    """[H, ncols] -> [128, NCH*ncols] with chunk c of rows at cols c*ncols."""
    return np.ascontiguousarray(
        w.reshape(NCH, 128, ncols).transpose(1, 0, 2).reshape(128, NCH * ncols))


def _make_in_maps(batch, input_ids, W_Q, W_K, W_V, b_Q, b_K, W_O):
    mdiag = np.broadcast_to(np.triu(np.ones((128, 128), np.float32)),
                            (2, 128, 128)).transpose(1, 0, 2)
    mdiag = _bf16(np.ascontiguousarray(mdiag))
    in_maps = []
    for core in range(NCORE):
        b, g = divmod(core, 4)
        base = 256 * g  # first feature column of this core's 4 heads
        wqq = np.stack([_chunked(W_Q[:, base + 128 * p: base + 128 * (p + 1)], 128)
                        for p in range(NPAIR)])
        wkk = np.stack([_chunked(W_K[:, base + 128 * p: base + 128 * (p + 1)], 128)
                        for p in range(NPAIR)])
        wv = _chunked(W_V[:, base: base + 256], 256)
        wo = np.ascontiguousarray(
            W_O[base: base + 256, :].reshape(2, 128, H)
            .transpose(1, 0, 2).reshape(128, 2 * H))
        bq = np.stack([b_Q[base + 128 * p: base + 128 * (p + 1)]
                       for p in range(NPAIR)], axis=1)
        bk = np.stack([b_K[base + 128 * p: base + 128 * (p + 1)]
                       for p in range(NPAIR)], axis=1)
        keep = input_ids[b] != 0
        nbias = np.where(keep, 0.0, NEG_BIAS).astype(np.float32)
        nbias = np.ascontiguousarray(nbias.reshape(NKC, 128).T)
        xt = np.ascontiguousarray(batch[b].T)
        in_maps.append({
            "xt": _bf16(xt), "wqq": _bf16(wqq), "wkk": _bf16(wkk),
            "wv": _bf16(wv), "wo": _bf16(wo),
            "bq": np.ascontiguousarray(bq), "bk": np.ascontiguousarray(bk),
            "nbias": nbias, "mdiag": mdiag,
        })
    return in_maps


def _run(in_maps, **kwargs):
    nc = _get_nc()
    return run_bass_kernel_spmd(nc, in_maps, core_ids=list(range(NCORE)), **kwargs)


def kernel(batch, input_ids, W_Q, W_K, W_V, b_Q, b_K, b_V, W_O, b_O,
           _results_out=None, **run_kwargs):
    batch = np.asarray(batch, np.float32)
    input_ids = np.asarray(input_ids)
    W_Q, W_K, W_V = (np.asarray(a, np.float32) for a in (W_Q, W_K, W_V))
    b_Q, b_K, b_V = (np.asarray(a, np.float32) for a in (b_Q, b_K, b_V))
    W_O = np.asarray(W_O, np.float32)
    b_O = np.asarray(b_O, np.float32)

    in_maps = _make_in_maps(batch, input_ids, W_Q, W_K, W_V, b_Q, b_K, W_O)
    res = _run(in_maps, **run_kwargs)
    if _results_out is not None:
        _results_out.append(res)
    ys = [np.asarray(res.results[c]["y"], np.float32) for c in range(NCORE)]
    out = np.stack([sum(ys[4 * b: 4 * b + 4]) for b in range(B)], axis=0)
    # exact fold: attn rows sum to 1, so the V bias passes through W_O
    bias = b_V @ W_O + b_O
    return (out + bias).astype(np.float32)


# revision 10
# speedup vs baseline: 1.3427x; 1.0253x over previous
"""Multi-head attention (B=2, S=2048, H=1024, NH=16, DK=DV=64) on 8 TRN2 cores.

Sharding: data-parallel over batch (2 groups of 4 cores) x tensor-parallel
over heads (4 heads per core).  Each core computes, for its batch sample and
its 4 heads:
    Q^T/K^T projections (features on partitions), V projection (natural),
    S^T = K @ Q^T per 128-key chunk (causal chunks only; the two heads of a
    pair run as concurrent row-tiled K=64 matmuls into one 2-bank PSUM tile),
    P^T = exp(S^T/8 + pad_bias)  (one ACTIVATE covers both heads),
    out^T = V_aug^T @ P^T  where V_aug = [V | ones] for even heads and
    [ones | V] for odd heads, so values and 1/denominator stay
    partition-aligned for both halves of attn^T,
    y_partial = attn^T.T @ W_O_rows   (row-sharded W_O).
Host sums the 4 bf16 partials per batch and adds b_V @ W_O + b_O (exact
fold of the V bias through the output projection).

The emission is hand-staged so the ACT engine (exp is the serial bottleneck,
~88us/core) starts ~12us in and never starves, while projection/output
matmuls fill the PE between attention chunks and keep the PE HAM-warm:

    S1   : pair-0 Q/K projections as an 8-bank PSUM wave (c-outer), paced by
           the x^T DMA stream
    S2-5 : per j: V-projection chunks for j's keys, then att(j, pair0)
    S6-7 : pair-1 Q/K projection groups woven between att(0..1, pair1)
    S8-9 : att(2..3, pair1) with W_O(0..2) woven into the chunk loops
    S10  : W_O(3)

Everything on the wide data path is bf16 (fp32 accumulation in PSUM).
Engine balance: PE matmuls only; ACT exps only; DVE does projection
writebacks, normalization, and y casts; GpSimd does the causal-diagonal
masks.  All PSUM pool scopes are arranged to stay within the 8 banks.
"""

import math
from contextlib import ExitStack

import numpy as np

import concourse.bass as bass
import concourse.mybir as mybir
from concourse import bacc
import concourse.tile as tile
from concourse.bass_utils import run_bass_kernel_spmd

F32 = mybir.dt.float32
BF16 = mybir.dt.bfloat16
EXP = mybir.ActivationFunctionType.Exp

B, S, H = 2, 2048, 1024
NH, DK, DV = 16, 64, 64
NCORE = 8
NCH = H // 128          # 8 contraction chunks over H
NJ = S // 512           # 4 query subtiles of 512
NKC = S // 128          # 16 key chunks
NPAIR = 2               # head pairs per core
SCALE = 1.0 / math.sqrt(DK)
NEG_BIAS = -30000.0     # exp(x + NEG_BIAS) == 0.0 in fp32 for any real score


def _emit(nc, d):
    with tile.TileContext(nc) as tc, ExitStack() as top:
        consts = top.enter_context(tc.tile_pool(name="consts", bufs=1))
        persist = top.enter_context(tc.tile_pool(name="persist", bufs=1))
        xtp = top.enter_context(tc.tile_pool(name="xtp", bufs=1))

        # ---- persistent activations ----
        qt_sb = []   # per pair: [128, S] bf16; rows 0:64 head A, 64:128 head B
        kt_sb = []
        attnT = []   # per pair: [128, S] bf16 normalized attn^T
        for p in range(NPAIR):
            qt_sb.append(persist.tile([128, S], BF16, tag=f"qt{p}", name=f"qt{p}sb"))
            kt_sb.append(persist.tile([128, S], BF16, tag=f"kt{p}", name=f"kt{p}sb"))
            attnT.append(persist.tile([128, S], BF16, tag=f"at{p}", name=f"at{p}sb"))
        # V_aug per head: [128 keys, NKC*128]; chunk t block is [V|ones] for
        # even heads, [ones|V] for odd heads.
        vaug = []
        for h in range(4):
            v = persist.tile([128, NKC * 128], BF16, tag=f"vaug{h}", name=f"vaug{h}sb")
            nc.vector.memset(v, 1.0)
            vaug.append(v)

        xt_sb = [xtp.tile([128, S], BF16, tag=f"xt{c}", name=f"xt{c}sb")
                 for c in range(NCH)]

        # Two DMA rings: sync carries pair-0 weights + the x^T stream (the
        # critical path to first matmul); the ACT ring carries everything
        # needed later, in parallel.
        wqq_sb = []
        wkk_sb = []
        for p in range(NPAIR):
            wqq_sb.append(consts.tile([128, NCH * 128], BF16, tag=f"wqq{p}",
                                      name=f"wqq{p}sb"))
            wkk_sb.append(consts.tile([128, NCH * 128], BF16, tag=f"wkk{p}",
                                      name=f"wkk{p}sb"))
        nc.sync.dma_start(out=wqq_sb[0], in_=d["wqq"][0])
        nc.sync.dma_start(out=wkk_sb[0], in_=d["wkk"][0])
        for c in range(NCH):
            nc.sync.dma_start(out=xt_sb[c], in_=d["xt"][c * 128:(c + 1) * 128, :])
        bq_sb = consts.tile([128, 2], F32, tag="bq", name="bqsb")
        nc.scalar.dma_start(out=bq_sb, in_=d["bq"][:])
        bk_sb = consts.tile([128, 2], F32, tag="bk", name="bksb")
        nc.scalar.dma_start(out=bk_sb, in_=d["bk"][:])
        nbias_sb = consts.tile([128, NKC], F32, tag="nbias", name="nbiassb")
        nc.scalar.dma_start(out=nbias_sb, in_=d["nbias"][:])
        wv_sb = consts.tile([128, NCH * 256], BF16, tag="wv", name="wvsb")
        nc.scalar.dma_start(out=wv_sb, in_=d["wv"][:])
        nc.scalar.dma_start(out=wqq_sb[1], in_=d["wqq"][1])
        nc.scalar.dma_start(out=wkk_sb[1], in_=d["wkk"][1])
        wo_sb = consts.tile([128, 2 * 1024], BF16, tag="wo", name="wosb")
        nc.scalar.dma_start(out=wo_sb, in_=d["wo"][:])
        mdiag_sb = consts.tile([128, 2, 128], BF16, tag="mdiag", name="mdiagsb")
        nc.gpsimd.dma_start(out=mdiag_sb, in_=d["mdiag"][:])

        # ---- S1: pair-0 Q/K projections, 8-bank wave paced by the xt DMA ----
        with tc.tile_pool(name="psqk8", bufs=1, space="PSUM") as psqk8:
            pss = {}
            for qk in range(2):
                for j in range(NJ):
                    pss[qk, j] = psqk8.tile([128, 512], F32, tag=f"pp{qk}{j}",
                                            name=f"pp{qk}{j}")
            for c in range(NCH):
                for qk, wsb in ((0, wqq_sb[0]), (1, wkk_sb[0])):
                    for j in range(NJ):
                        nc.tensor.matmul(
                            pss[qk, j],
                            wsb[:, c * 128:(c + 1) * 128],
                            xt_sb[c][:, j * 512:(j + 1) * 512],
                            start=(c == 0), stop=(c == NCH - 1),
                        )
            # writebacks on ACT (idle here); j-major so att(0,0) unblocks first
            for j in range(NJ):
                for qk, (bsb, dst) in enumerate(((bq_sb, qt_sb[0]),
                                                 (bk_sb, kt_sb[0]))):
                    nc.scalar.activation(
                        dst[:, j * 512:(j + 1) * 512], pss[qk, j],
                        mybir.ActivationFunctionType.Identity,
                        bias=bsb[:, 0:1])

        with (
            tc.tile_pool(name="psab", bufs=2, space="PSUM") as psab,
            tc.tile_pool(name="pso", bufs=1, space="PSUM") as psop,
            tc.tile_pool(name="ptp", bufs=3) as ptp,
            tc.tile_pool(name="nrm", bufs=2) as nrm,
            tc.tile_pool(name="ysb", bufs=4) as ysb,
        ):
            def qkproj_group(pool, p, qk, j):
                """Pair-1 style single-tile projection group (c-inner)."""
                wsb = (wqq_sb, wkk_sb)[qk][p]
                bsb = (bq_sb, bk_sb)[qk]
                dst = (qt_sb, kt_sb)[qk][p]
                ps = pool.tile([128, 512], F32, tag="sc", name=f"qk{p}{qk}{j}")
                for c in range(NCH):
                    nc.tensor.matmul(
                        ps, wsb[:, c * 128:(c + 1) * 128],
                        xt_sb[c][:, j * 512:(j + 1) * 512],
                        start=(c == 0), stop=(c == NCH - 1),
                    )
                # writeback on ACT: it slots between exps right where the
                # dependent scores need it, with no DVE-queue latency
                nc.scalar.activation(
                    dst[:, j * 512:(j + 1) * 512], ps,
                    mybir.ActivationFunctionType.Identity,
                    bias=bsb[:, p:p + 1])

            def vproj(pool, t):
                ps = pool.tile([128, 256], F32, tag="v", name=f"psv{t}")
                for c in range(NCH):
                    nc.tensor.matmul(
                        ps, xt_sb[c][:, t * 128:(t + 1) * 128],
                        wv_sb[:, c * 256:(c + 1) * 256],
                        start=(c == 0), stop=(c == NCH - 1),
                    )
                for h in range(4):
                    off = t * 128 + (0 if h % 2 == 0 else 64)
                    nc.vector.tensor_copy(vaug[h][:, off:off + 64],
                                          ps[:, h * 64:(h + 1) * 64])

            def att(j, p, weave=None):
                """Attention for (j, pair p): scores/exp/mask/PV + normalize.
                weave: optional {chunk_index: fn} emitted after that chunk."""
                hA, hB = 2 * p, 2 * p + 1
                oA = psop.tile([128, 512], F32, tag="oA", bufs=1, name=f"oA{p}{j}")
                oB = psop.tile([128, 512], F32, tag="oB", bufs=1, name=f"oB{p}{j}")
                cmax = 4 * j + 3
                for c in range(cmax + 1):
                    t = c - 4 * j
                    fo = 128 * t if t > 0 else 0
                    w = 512 - fo
                    qsl = slice(j * 512 + fo, (j + 1) * 512)
                    ksl = slice(c * 128, (c + 1) * 128)
                    sAB = psab.tile([128, 2, 512], F32, tag="sAB",
                                    name=f"sAB{p}{j}{c}")
                    nc.tensor.matmul(
                        sAB[:, 0, :w], kt_sb[p][0:64, ksl],
                        qt_sb[p][0:64, qsl], start=True, stop=True)
                    nc.tensor.matmul(
                        sAB[:, 1, :w], kt_sb[p][64:128, ksl],
                        qt_sb[p][64:128, qsl], start=True, stop=True)
                    pAB = ptp.tile([128, 2, 512], BF16, tag="pAB",
                                   name=f"pAB{p}{j}{c}")
                    if w == 512:
                        nc.scalar.activation(pAB[:, :, :], sAB[:, :, :], EXP,
                                             bias=nbias_sb[:, c:c + 1],
                                             scale=SCALE)
                    else:
                        nc.scalar.activation(pAB[:, :, :w], sAB[:, :, :w], EXP,
                                             bias=nbias_sb[:, c:c + 1],
                                             scale=SCALE)
                    if t >= 0:
                        # diagonal 128x128 block: zero keys below the diagonal
                        nc.gpsimd.tensor_mul(pAB[:, :, 0:128],
                                             pAB[:, :, 0:128], mdiag_sb)
                    nc.tensor.matmul(
                        oA[:, fo:512], vaug[hA][:, ksl], pAB[:, 0, :w],
                        start=(c == 0), stop=(c == cmax))
                    nc.tensor.matmul(
                        oB[:, fo:512], vaug[hB][:, ksl], pAB[:, 1, :w],
                        start=(c == 0), stop=(c == cmax))
                    if weave and c in weave:
                        weave[c]()

                # normalize: head A denom at oA[64:128], head B denom at
                # oB[0:64]; one cross DMA each makes values/recip lane-aligned.
                jsl = slice(j * 512, (j + 1) * 512)
                scr = nrm.tile([128, 512], F32, tag="scr", name=f"scr{p}{j}")
                nc.vector.tensor_copy(scr[64:128, :], oA[64:128, :])
                nc.vector.tensor_copy(scr[0:64, :], oB[0:64, :])
                rec = nrm.tile([128, 512], F32, tag="rec", name=f"rec{p}{j}")
                nc.sync.dma_start(out=rec[0:64, :], in_=scr[64:128, :])
                nc.sync.dma_start(out=rec[64:128, :], in_=scr[0:64, :])
                nc.vector.reciprocal_approx_fast(out=rec, in_=rec)
                nc.vector.tensor_mul(attnT[p][0:64, jsl], oA[0:64, :],
                                     rec[0:64, :])
                nc.vector.tensor_mul(attnT[p][64:128, jsl], oB[64:128, :],
                                     rec[64:128, :])

            def wo_tile(pool, q, half, cast_eng=None):
                pf = pool.tile([128, 512], F32, tag="sc", name=f"pf{q}{half}")
                for p in range(NPAIR):
                    nc.tensor.matmul(
                        pf, attnT[p][:, q * 128:(q + 1) * 128],
                        wo_sb[:, p * 1024 + half * 512:
                              p * 1024 + half * 512 + 512],
                        start=(p == 0), stop=(p == 1),
                    )
                yt = ysb.tile([128, 512], BF16, tag="y", name=f"yt{q}{half}")
                if cast_eng == "scalar":
                    nc.scalar.activation(yt, pf,
                                         mybir.ActivationFunctionType.Copy)
                else:
                    nc.vector.tensor_copy(yt, pf)
                nc.sync.dma_start(
                    out=d["y"][q * 128:(q + 1) * 128,
                               half * 512:(half + 1) * 512],
                    in_=yt)

            # ---- S2-S5: V projection woven into att(*, pair0) chunk loops ----
            with tc.tile_pool(name="psv", bufs=2, space="PSUM") as psv:
                for t in range(4):
                    vproj(psv, t)
                att(0, 0)
                for j in range(1, NJ):
                    att(j, 0, weave={i: (lambda t=4 * j + i: vproj(psv, t))
                                     for i in range(4)})

            # ---- S6-S10: pair-1 projections + W_O woven into att(*, pair1);
            # qk-group and W_O psum tiles share one 2-bank rotating pool ----
            with tc.tile_pool(name="ps2", bufs=2, space="PSUM") as ps2:
                def qkg(qk, j):
                    return lambda: qkproj_group(ps2, 1, qk, j)

                def wot(q, half, cast_eng=None):
                    return lambda: wo_tile(ps2, q, half, cast_eng)

                qkproj_group(ps2, 1, 0, 0)
                qkproj_group(ps2, 1, 1, 0)
                att(0, 1, weave={0: qkg(0, 1), 2: qkg(1, 1)})
                # WO tiles woven one per chunk, each a stage behind its
                # normalize so the PE never waits on the DVE chain
                att(1, 1, weave={1: wot(0, 0), 2: wot(0, 1), 3: qkg(0, 2),
                                 4: wot(1, 0), 5: qkg(1, 2), 6: wot(1, 1),
                                 7: wot(2, 0)})
                att(2, 1, weave={0: wot(2, 1), 1: wot(3, 0), 2: qkg(0, 3),
                                 3: wot(3, 1), 4: wot(4, 0), 6: qkg(1, 3),
                                 7: wot(4, 1), 8: wot(5, 0), 9: wot(5, 1),
                                 10: wot(6, 0), 11: wot(6, 1)})
                att(3, 1, weave={0: wot(7, 0), 1: wot(7, 1), 4: wot(8, 0),
                                 5: wot(8, 1), 7: wot(9, 0), 8: wot(9, 1),
                                 10: wot(10, 0), 11: wot(10, 1),
                                 13: wot(11, 0), 14: wot(11, 1)})
                for q in range(12, 16):
                    wo_tile(ps2, q, 0, "scalar")
                    wo_tile(ps2, q, 1, None)


_NC_CACHE = {}


def _get_nc():
    if "nc" not in _NC_CACHE:
        nc = bacc.Bacc(None, target_bir_lowering=False)
        d = {
            "xt": nc.dram_tensor("xt", [H, S], BF16, kind="ExternalInput"),
            "wqq": nc.dram_tensor("wqq", [NPAIR, 128, NCH * 128], BF16,
                                  kind="ExternalInput"),
            "wkk": nc.dram_tensor("wkk", [NPAIR, 128, NCH * 128], BF16,
                                  kind="ExternalInput"),
            "wv": nc.dram_tensor("wv", [128, NCH * 256], BF16,
                                 kind="ExternalInput"),
            "wo": nc.dram_tensor("wo", [128, 2 * 1024], BF16,
                                 kind="ExternalInput"),
            "bq": nc.dram_tensor("bq", [128, 2], F32, kind="ExternalInput"),
            "bk": nc.dram_tensor("bk", [128, 2], F32, kind="ExternalInput"),
            "nbias": nc.dram_tensor("nbias", [128, NKC], F32,
                                    kind="ExternalInput"),
            "mdiag": nc.dram_tensor("mdiag", [128, 2, 128], BF16,
                                    kind="ExternalInput"),
            "y": nc.dram_tensor("y", [S, H], BF16, kind="ExternalOutput"),
        }
        _emit(nc, d)
        nc.finalize()
        _NC_CACHE["nc"] = nc
    return _NC_CACHE["nc"]


def _bf16(a):
    import ml_dtypes
    return np.ascontiguousarray(a.astype(ml_dtypes.bfloat16))


def _chunked(w, ncols):
    """[H, ncols] -> [128, NCH*ncols] with chunk c of rows at cols c*ncols."""
    return np.ascontiguousarray(
        w.reshape(NCH, 128, ncols).transpose(1, 0, 2).reshape(128, NCH * ncols))


def _make_in_maps(batch, input_ids, W_Q, W_K, W_V, b_Q, b_K, W_O):
    mdiag = np.broadcast_to(np.triu(np.ones((128, 128), np.float32)),
                            (2, 128, 128)).transpose(1, 0, 2)
    mdiag = _bf16(np.ascontiguousarray(mdiag))
    in_maps = []
    for core in range(NCORE):
        b, g = divmod(core, 4)
        base = 256 * g  # first feature column of this core's 4 heads
        wqq = np.stack([_chunked(W_Q[:, base + 128 * p: base + 128 * (p + 1)], 128)
                        for p in range(NPAIR)])
        wkk = np.stack([_chunked(W_K[:, base + 128 * p: base + 128 * (p + 1)], 128)
                        for p in range(NPAIR)])
        wv = _chunked(W_V[:, base: base + 256], 256)
        wo = np.ascontiguousarray(
            W_O[base: base + 256, :].reshape(2, 128, H)
            .transpose(1, 0, 2).reshape(128, 2 * H))
        bq = np.stack([b_Q[base + 128 * p: base + 128 * (p + 1)]
                       for p in range(NPAIR)], axis=1)
        bk = np.stack([b_K[base + 128 * p: base + 128 * (p + 1)]
                       for p in range(NPAIR)], axis=1)
        keep = input_ids[b] != 0
        nbias = np.where(keep, 0.0, NEG_BIAS).astype(np.float32)
        nbias = np.ascontiguousarray(nbias.reshape(NKC, 128).T)
        xt = np.ascontiguousarray(batch[b].T)
        in_maps.append({
            "xt": _bf16(xt), "wqq": _bf16(wqq), "wkk": _bf16(wkk),
            "wv": _bf16(wv), "wo": _bf16(wo),
            "bq": np.ascontiguousarray(bq), "bk": np.ascontiguousarray(bk),
            "nbias": nbias, "mdiag": mdiag,
        })
    return in_maps


def _run(in_maps, **kwargs):
    nc = _get_nc()
    return run_bass_kernel_spmd(nc, in_maps, core_ids=list(range(NCORE)), **kwargs)


def kernel(batch, input_ids, W_Q, W_K, W_V, b_Q, b_K, b_V, W_O, b_O,
           _results_out=None, **run_kwargs):
    batch = np.asarray(batch, np.float32)
    input_ids = np.asarray(input_ids)
    W_Q, W_K, W_V = (np.asarray(a, np.float32) for a in (W_Q, W_K, W_V))
    b_Q, b_K, b_V = (np.asarray(a, np.float32) for a in (b_Q, b_K, b_V))
    W_O = np.asarray(W_O, np.float32)
    b_O = np.asarray(b_O, np.float32)

    in_maps = _make_in_maps(batch, input_ids, W_Q, W_K, W_V, b_Q, b_K, W_O)
    res = _run(in_maps, **run_kwargs)
    if _results_out is not None:
        _results_out.append(res)
    ys = [np.asarray(res.results[c]["y"], np.float32) for c in range(NCORE)]
    out = np.stack([sum(ys[4 * b: 4 * b + 4]) for b in range(B)], axis=0)
    # exact fold: attn rows sum to 1, so the V bias passes through W_O
    bias = b_V @ W_O + b_O
    return (out + bias).astype(np.float32)
